# revision 13
# baseline (speedup 1.0000x reference)
"""Bass/Trainium2 kernel for nn_GroundingLoss (symmetric token-level InfoNCE).

Math (matches the jax reference):
    sim[a,b,i,j] = sum_k x[a,i,k] * z[b,j,k]
    S[a,b]       = (1/J) * sum_j  [ sum_i softmax_i(sim[a,b,:,j]) * sim[a,b,:,j] ]
    loss         = mean( logsumexp_a(S) - diag + logsumexp_b(S) - diag )

Sharding: the batch axis of x (a) is split across the 8 cores; every core
computes S[a_local, :] against all of z.

Device pipeline (v6), per atile t (4 a's x 32 i = 128 partitions) and
chunk-group cg (1024 (b,j) columns):
  PE   : fp8(e4m3) DoubleRow matmul — the whole K=256 contraction in one
         matmul — sim fp32 into PSUM [128, 1024].
  ACT  : u = Copy(sim + 30) PSUM->SBUF bf16 (the only engine that can
         drain PSUM fast; one FD=1024 op per atile).
  DVE  : e_bits(uint16) = max(A*u, 0), A = 128/ln2, over a whole-cg tile
         [128, 8192] at 4x — the uint16 bit pattern read as bf16 is
         ~exp(u)*2^-127 (Schraudolph; clamp-to-0 = exact softmax drop).
         es = u * e at 2x.
  PE   : num'' = sum_i es, den = sum_i e via block-diag ones matmuls running
         concurrently in FOUR PE column groups (tile_position cols 0/32/64/96)
         accumulating over all 8 atiles into one PSUM bank [128, 512]
         (partition quarters = num/den x column-half).  Emitted one cg late
         so they never head-of-line block the PE queue.
  DVE  : copy nd PSUM->SBUF [128, 512]; DMA out.
Host: r = num/den - 30 (shift and 2^-127 cancel exactly), S = mean_j r,
tiny [256,256] logsumexp epilogue.
"""

import numpy as np

N, I, J, K = 256, 32, 32, 256
NCORES = 8
NL = N // NCORES          # 32 local a's per core
BJ = N * J                # 8192 (b, j) pairs
CG = 1024                 # free elements per chunk-group (32 b's x 32 j's)
NCG = BJ // CG            # 8
NAT = NL // 4             # 8 atiles of (4 a's x 32 i's) = 128 partitions
TSHIFT = -30.0            # sim in [-101, 86]
SCHRAUD_A = 184.6650085   # 128 / ln(2)
ACT_BIAS = 184.6650085 * 30.0

_cached = None


def _build():
    import concourse.bacc as bacc
    import concourse.mybir as mybir
    import concourse.tile as tile

    f32 = mybir.dt.float32
    bf16 = mybir.dt.bfloat16
    i16 = mybir.dt.int16
    u16 = mybir.dt.uint16
    f8 = mybir.dt.float8e4
    AF_T = mybir.ActivationFunctionType
    ALU = mybir.AluOpType
    DR = mybir.MatmulPerfMode.DoubleRow

    nc = bacc.Bacc("TRN2", target_bir_lowering=False, debug=False)
    xt_d = nc.dram_tensor("xt", [128, NAT, 2, 128], f8, kind="ExternalInput").ap()
    zt_d = nc.dram_tensor("zt", [128, NCG, 2, CG], f8, kind="ExternalInput").ap()
    on_d = nc.dram_tensor("ones", [128, NAT * NL], bf16, kind="ExternalInput").ap()
    out_d = nc.dram_tensor("out", [64, BJ], f32, kind="ExternalOutput").ap()

    with tile.TileContext(nc) as tc:
        with (
            tc.tile_pool(name="const", bufs=1) as cpool,
            tc.tile_pool(name="psum", bufs=3, space="PSUM") as ppool,
            tc.tile_pool(name="nd", bufs=1, space="PSUM") as ndpool,
            tc.tile_pool(name="sb", bufs=2) as spool,
            tc.tile_pool(name="ob", bufs=2) as opool,
        ):
            xt = cpool.tile([128, NAT, 2, 128], f8)
            zt = cpool.tile([128, NCG, 2, CG], f8)
            ones = cpool.tile([128, NAT * NL], bf16)
            # parallel queues: gpsimd carries zt[0], sync carries xt, so the
            # first mains' inputs land together; the rest alternate.
            nc.gpsimd.dma_start(zt[:, 0], zt_d[:, 0])
            nc.sync.dma_start(xt[:], xt_d[:])
            nc.gpsimd.dma_start(ones[:], on_d[:, :])
            for q in range(1, NCG):
                eng = nc.sync if q % 2 else nc.gpsimd
                eng.dma_start(zt[:, q], zt_d[:, q])

            e_tiles = {}
            # DIRECT cgs: DVE builds e straight from PSUM (2-op tensor_scalar,
            # uint16 saturation clamps) — offloads the ACT engine; their num
            # comes out scaled by A (es = (A*u)*e), fixed on host.
            DIRECT = ()
            GPS_TT = ()

            def emit_mains(cg, t, u):
                sim = ppool.tile([128, CG], f32, tag="sim", name=f"sim_{cg}_{t}")
                lhsT = xt[:, t]                       # [128, 2, 128] fp8
                for h in range(2):
                    rhs = zt[:, cg, :, h * 512 : (h + 1) * 512]
                    nc.tensor.matmul(
                        sim[:, h * 512 : (h + 1) * 512],
                        lhsT, rhs, start=True, stop=True, perf_mode=DR,
                    )
                nc.scalar.activation(
                    u[:, t * CG : (t + 1) * CG], sim[:],
                    AF_T.Copy, bias=-TSHIFT, scale=1.0,
                )

            def emit_half(cg, half, u, e, es):
                hs = slice(half * 4 * CG, (half + 1) * 4 * CG)
                nc.vector.tensor_scalar(
                    e.bitcast(i16)[:, hs], u[:, hs], SCHRAUD_A, 0.0, ALU.mult, ALU.max
                )
                nc.vector.tensor_mul(es[:, hs], u[:, hs], e.bitcast(bf16)[:, hs])

            def emit_reds(cg, t, e, es, nd):
                eb = e.bitcast(bf16)
                onesT = ones[:, t * NL : (t + 1) * NL]
                st, sp = (t == 0), (t == NAT - 1)
                for h in range(2):
                    src = slice(t * CG + h * 512, t * CG + (h + 1) * 512)
                    dst = slice(h * 512, (h + 1) * 512)
                    nc.tensor.matmul(nd[0:32, dst], onesT, es[:, src], start=st, stop=sp)
                    nc.tensor.matmul(nd[32:64, dst], onesT, eb[:, src], start=st, stop=sp)

            def emit_out(cg, nd):
                ob = opool.tile([64, CG], f32, tag="ob", name=f"ob_{cg}")
                nc.vector.tensor_copy(ob[:], nd[:])
                nc.sync.dma_start(out_d[:, cg * CG : (cg + 1) * CG], ob[:])

            tiles = {}
            nd_cur = [None]

            def front_step(cg, t):
                if t == 0:
                    tiles[cg] = (
                        spool.tile([128, NAT * CG], bf16, tag="u", name=f"u_{cg}"),
                        spool.tile([128, NAT * CG], u16, tag="e", name=f"e_{cg}", bufs=2),
                        spool.tile([128, NAT * CG], bf16, tag="es", name=f"es_{cg}", bufs=2),
                    )
                u, e, es = tiles[cg]
                emit_mains(cg, t, u)
                if t == 3 or t == NAT - 1:
                    emit_half(cg, t // 4, u, e, es)

            def back_step(cg, t):
                u, e, es = tiles[cg]
                if t == 0:
                    nd_cur[0] = ndpool.tile([64, CG], f32, tag="nd", name=f"nd_{cg}")
                emit_reds(cg, t, e, es, nd_cur[0])
                if t == NAT - 1:
                    emit_out(cg, nd_cur[0])
                    del tiles[cg]

            # reds trail the fronts by DELAY t-slots (enough for the es half
            # they read to exist), clumped in pairs to halve PE
            # mains<->reds transitions.
            DELAY = 5
            slots = [(cg, t) for cg in range(NCG) for t in range(NAT)]
            for idx in range(len(slots) + DELAY):
                if idx < len(slots):
                    front_step(*slots[idx])
                r = idx - DELAY
                if 0 <= r < len(slots) and (r % 2 == 1 or r == len(slots) - 1):
                    back_step(*slots[r - 1])
                    if r % 2 == 1:
                        back_step(*slots[r])
    nc.compile()
    return nc


def _prep_inputs(x, z):
    import ml_dtypes

    bf = ml_dtypes.bfloat16
    f8 = ml_dtypes.float8_e4m3
    x8 = np.ascontiguousarray(x, dtype=np.float32).astype(f8)
    z8 = np.ascontiguousarray(z, dtype=np.float32).astype(f8)
    # zt[p, cg, kc, u] = z[b(cg,u), j(u), 128*kc + p]
    zt0 = z8.transpose(2, 0, 1).reshape(K, BJ)          # [k, (b,j)]
    zt = np.empty((128, NCG, 2, CG), dtype=f8)
    for cg in range(NCG):
        cs = slice(cg * CG, (cg + 1) * CG)
        zt[:, cg, 0, :] = zt0[0:128, cs]
        zt[:, cg, 1, :] = zt0[128:256, cs]
    zt = np.ascontiguousarray(zt)
    # block-diagonal ones: ones[p, t*NL + 4t + p//32] = 1
    on = np.zeros((128, NAT * NL), dtype=bf)
    for t in range(NAT):
        for p in range(128):
            on[p, t * NL + 4 * t + p // 32] = 1
    in_maps = []
    for d in range(NCORES):
        xl = x8[d * NL : (d + 1) * NL]                  # [NL, I, K]
        xf = xl.transpose(2, 0, 1).reshape(K, NL * I)   # [k, (a,i)]
        xt = np.empty((128, NAT, 2, 128), dtype=f8)
        for t in range(NAT):
            xt[:, t, 0, :] = xf[0:128, t * 128 : (t + 1) * 128]
            xt[:, t, 1, :] = xf[128:256, t * 128 : (t + 1) * 128]
        in_maps.append({"xt": np.ascontiguousarray(xt), "zt": zt, "ones": on})
    return in_maps


def _epilogue(results):
    S = np.empty((N, N), dtype=np.float64)
    A = float(SCHRAUD_A)
    nscale = np.ones(BJ).reshape(N, J)
    for d in range(NCORES):
        arr = results[d]["out"].astype(np.float64)     # [64, BJ]
        num = arr[0:32].reshape(NL, N, J)
        den = arr[32:64].reshape(NL, N, J)
        r = num * nscale[None] / den + TSHIFT
        S[d * NL : (d + 1) * NL, :] = r.mean(axis=2)
    diag = np.diagonal(S)
    m0 = S.max(axis=0)
    lx = m0 + np.log(np.exp(S - m0[None, :]).sum(axis=0)) - diag
    m1 = S.max(axis=1)
    lz = m1 + np.log(np.exp(S - m1[:, None]).sum(axis=1)) - diag
    loss = (lx + lz).mean()
    return np.asarray(loss, dtype=np.float32)


def run_on_device(x, z, trace=False):
    """Returns (loss, BassKernelResults)."""
    from concourse.bass_utils import run_bass_kernel_spmd

    global _cached
    if _cached is None:
        _cached = _build()
    nc = _cached
    in_maps = _prep_inputs(x, z)
    res = run_bass_kernel_spmd(nc, in_maps, list(range(NCORES)), trace=trace)
    return _epilogue(res.results), res


def kernel(x, z):
    loss, _ = run_on_device(x, z)
    return loss


# revision 14
# speedup vs baseline: 1.0146x; 1.0146x over previous
"""Bass/Trainium2 kernel for nn_GroundingLoss (symmetric token-level InfoNCE).

Math (matches the jax reference):
    sim[a,b,i,j] = sum_k x[a,i,k] * z[b,j,k]
    S[a,b]       = (1/J) * sum_j  [ sum_i softmax_i(sim[a,b,:,j]) * sim[a,b,:,j] ]
    loss         = mean( logsumexp_a(S) - diag + logsumexp_b(S) - diag )

Sharding: the batch axis of x (a) is split across the 8 cores; every core
computes S[a_local, :] against all of z.

Device pipeline (v6), per atile t (4 a's x 32 i = 128 partitions) and
chunk-group cg (1024 (b,j) columns):
  PE   : fp8(e4m3) DoubleRow matmul — the whole K=256 contraction in one
         matmul — sim fp32 into PSUM [128, 1024].
  ACT  : u = Copy(sim + 30) PSUM->SBUF bf16 (the only engine that can
         drain PSUM fast; one FD=1024 op per atile).
  DVE  : e_bits(uint16) = max(A*u, 0), A = 128/ln2, over a whole-cg tile
         [128, 8192] at 4x — the uint16 bit pattern read as bf16 is
         ~exp(u)*2^-127 (Schraudolph; clamp-to-0 = exact softmax drop).
         es = u * e at 2x.
  PE   : num'' = sum_i es, den = sum_i e via block-diag ones matmuls running
         concurrently in FOUR PE column groups (tile_position cols 0/32/64/96)
         accumulating over all 8 atiles into one PSUM bank [128, 512]
         (partition quarters = num/den x column-half).  Emitted one cg late
         so they never head-of-line block the PE queue.
  DVE  : copy nd PSUM->SBUF [128, 512]; DMA out.
Host: r = num/den - 30 (shift and 2^-127 cancel exactly), S = mean_j r,
tiny [256,256] logsumexp epilogue.
"""

import numpy as np

N, I, J, K = 256, 32, 32, 256
NCORES = 8
NL = N // NCORES          # 32 local a's per core
BJ = N * J                # 8192 (b, j) pairs
CG = 1024                 # free elements per chunk-group (32 b's x 32 j's)
NCG = BJ // CG            # 8
NAT = NL // 4             # 8 atiles of (4 a's x 32 i's) = 128 partitions
TSHIFT = -30.0            # sim in [-101, 86]
SCHRAUD_A = 184.6650085   # 128 / ln(2)
ACT_BIAS = 184.6650085 * 30.0

_cached = None


def _build():
    import concourse.bacc as bacc
    import concourse.mybir as mybir
    import concourse.tile as tile

    f32 = mybir.dt.float32
    bf16 = mybir.dt.bfloat16
    i16 = mybir.dt.int16
    u16 = mybir.dt.uint16
    f8 = mybir.dt.float8e4
    AF_T = mybir.ActivationFunctionType
    ALU = mybir.AluOpType
    DR = mybir.MatmulPerfMode.DoubleRow

    nc = bacc.Bacc("TRN2", target_bir_lowering=False, debug=False)
    xt_d = nc.dram_tensor("xt", [128, NAT, 2, 128], f8, kind="ExternalInput").ap()
    zt_d = nc.dram_tensor("zt", [128, NCG, 2, CG], f8, kind="ExternalInput").ap()
    on_d = nc.dram_tensor("ones", [128, NAT * NL], bf16, kind="ExternalInput").ap()
    out_d = nc.dram_tensor("out", [64, BJ], f32, kind="ExternalOutput").ap()

    with tile.TileContext(nc) as tc:
        with (
            tc.tile_pool(name="const", bufs=1) as cpool,
            tc.tile_pool(name="psum", bufs=3, space="PSUM") as ppool,
            tc.tile_pool(name="nd", bufs=1, space="PSUM") as ndpool,
            tc.tile_pool(name="sb", bufs=2) as spool,
            tc.tile_pool(name="ob", bufs=2) as opool,
        ):
            xt = cpool.tile([128, NAT, 2, 128], f8)
            zt = cpool.tile([128, NCG, 2, CG], f8)
            ones = cpool.tile([128, NAT * NL], bf16)
            nc.sync.dma_start(zt[:, 0], zt_d[:, 0])
            nc.sync.dma_start(xt[:], xt_d[:])
            nc.sync.dma_start(ones[:], on_d[:, :])
            for q in range(1, NCG):
                nc.sync.dma_start(zt[:, q], zt_d[:, q])

            e_tiles = {}
            # DIRECT cgs: DVE builds e straight from PSUM (2-op tensor_scalar,
            # uint16 saturation clamps) — offloads the ACT engine; their num
            # comes out scaled by A (es = (A*u)*e), fixed on host.
            DIRECT = ()
            GPS_TT = ()

            def emit_mains(cg, t, u):
                sim = ppool.tile([128, CG], f32, tag="sim", name=f"sim_{cg}_{t}")
                lhsT = xt[:, t]                       # [128, 2, 128] fp8
                for h in range(2):
                    rhs = zt[:, cg, :, h * 512 : (h + 1) * 512]
                    nc.tensor.matmul(
                        sim[:, h * 512 : (h + 1) * 512],
                        lhsT, rhs, start=True, stop=True, perf_mode=DR,
                    )
                nc.scalar.activation(
                    u[:, t * CG : (t + 1) * CG], sim[:],
                    AF_T.Copy, bias=-TSHIFT, scale=1.0,
                )

            def emit_half(cg, half, u, e, es):
                hs = slice(half * 4 * CG, (half + 1) * 4 * CG)
                nc.vector.tensor_scalar(
                    e.bitcast(i16)[:, hs], u[:, hs], SCHRAUD_A, 0.0, ALU.mult, ALU.max
                )
                nc.vector.tensor_mul(es[:, hs], u[:, hs], e.bitcast(bf16)[:, hs])

            def emit_reds(cg, t, e, es, nd):
                eb = e.bitcast(bf16)
                onesT = ones[:, t * NL : (t + 1) * NL]
                st, sp = (t == 0), (t == NAT - 1)
                for h in range(2):
                    src = slice(t * CG + h * 512, t * CG + (h + 1) * 512)
                    dst = slice(h * 512, (h + 1) * 512)
                    nc.tensor.matmul(nd[0:32, dst], onesT, es[:, src], start=st, stop=sp)
                    nc.tensor.matmul(nd[32:64, dst], onesT, eb[:, src], start=st, stop=sp)

            def emit_out(cg, nd):
                ob = opool.tile([64, CG], f32, tag="ob", name=f"ob_{cg}")
                nc.vector.tensor_copy(ob[:], nd[:])
                nc.sync.dma_start(out_d[:, cg * CG : (cg + 1) * CG], ob[:])

            tiles = {}
            nd_cur = [None]

            def front_step(cg, t):
                if t == 0:
                    tiles[cg] = (
                        spool.tile([128, NAT * CG], bf16, tag="u", name=f"u_{cg}"),
                        spool.tile([128, NAT * CG], u16, tag="e", name=f"e_{cg}", bufs=2),
                        spool.tile([128, NAT * CG], bf16, tag="es", name=f"es_{cg}", bufs=2),
                    )
                u, e, es = tiles[cg]
                emit_mains(cg, t, u)
                if t == 3 or t == NAT - 1:
                    emit_half(cg, t // 4, u, e, es)

            def back_step(cg, t):
                u, e, es = tiles[cg]
                if t == 0:
                    nd_cur[0] = ndpool.tile([64, CG], f32, tag="nd", name=f"nd_{cg}")
                emit_reds(cg, t, e, es, nd_cur[0])
                if t == NAT - 1:
                    emit_out(cg, nd_cur[0])
                    del tiles[cg]

            # reds trail the fronts by DELAY t-slots (enough for the es half
            # they read to exist), clumped in pairs to halve PE
            # mains<->reds transitions.
            DELAY = 5
            slots = [(cg, t) for cg in range(NCG) for t in range(NAT)]
            for idx in range(len(slots) + DELAY):
                if idx < len(slots):
                    front_step(*slots[idx])
                r = idx - DELAY
                if 0 <= r < len(slots) and (r % 2 == 1 or r == len(slots) - 1):
                    back_step(*slots[r - 1])
                    if r % 2 == 1:
                        back_step(*slots[r])
    nc.compile()
    return nc


def _prep_inputs(x, z):
    import ml_dtypes

    bf = ml_dtypes.bfloat16
    f8 = ml_dtypes.float8_e4m3
    x8 = np.ascontiguousarray(x, dtype=np.float32).astype(f8)
    z8 = np.ascontiguousarray(z, dtype=np.float32).astype(f8)
    # zt[p, cg, kc, u] = z[b(cg,u), j(u), 128*kc + p]
    zt0 = z8.transpose(2, 0, 1).reshape(K, BJ)          # [k, (b,j)]
    zt = np.empty((128, NCG, 2, CG), dtype=f8)
    for cg in range(NCG):
        cs = slice(cg * CG, (cg + 1) * CG)
        zt[:, cg, 0, :] = zt0[0:128, cs]
        zt[:, cg, 1, :] = zt0[128:256, cs]
    zt = np.ascontiguousarray(zt)
    # block-diagonal ones: ones[p, t*NL + 4t + p//32] = 1
    on = np.zeros((128, NAT * NL), dtype=bf)
    for t in range(NAT):
        for p in range(128):
            on[p, t * NL + 4 * t + p // 32] = 1
    in_maps = []
    for d in range(NCORES):
        xl = x8[d * NL : (d + 1) * NL]                  # [NL, I, K]
        xf = xl.transpose(2, 0, 1).reshape(K, NL * I)   # [k, (a,i)]
        xt = np.empty((128, NAT, 2, 128), dtype=f8)
        for t in range(NAT):
            xt[:, t, 0, :] = xf[0:128, t * 128 : (t + 1) * 128]
            xt[:, t, 1, :] = xf[128:256, t * 128 : (t + 1) * 128]
        in_maps.append({"xt": np.ascontiguousarray(xt), "zt": zt, "ones": on})
    return in_maps


def _epilogue(results):
    S = np.empty((N, N), dtype=np.float64)
    A = float(SCHRAUD_A)
    nscale = np.ones(BJ).reshape(N, J)
    for d in range(NCORES):
        arr = results[d]["out"].astype(np.float64)     # [64, BJ]
        num = arr[0:32].reshape(NL, N, J)
        den = arr[32:64].reshape(NL, N, J)
        r = num * nscale[None] / den + TSHIFT
        S[d * NL : (d + 1) * NL, :] = r.mean(axis=2)
    diag = np.diagonal(S)
    m0 = S.max(axis=0)
    lx = m0 + np.log(np.exp(S - m0[None, :]).sum(axis=0)) - diag
    m1 = S.max(axis=1)
    lz = m1 + np.log(np.exp(S - m1[:, None]).sum(axis=1)) - diag
    loss = (lx + lz).mean()
    return np.asarray(loss, dtype=np.float32)


def run_on_device(x, z, trace=False):
    """Returns (loss, BassKernelResults)."""
    from concourse.bass_utils import run_bass_kernel_spmd

    global _cached
    if _cached is None:
        _cached = _build()
    nc = _cached
    in_maps = _prep_inputs(x, z)
    res = run_bass_kernel_spmd(nc, in_maps, list(range(NCORES)), trace=trace)
    return _epilogue(res.results), res


def kernel(x, z):
    loss, _ = run_on_device(x, z)
    return loss
